# revision 2
# baseline (speedup 1.0000x reference)
"""Cross-attention kernel for 8 TRN2 NeuronCores (Bass/Tile, SPMD).

Problem (hardcoded): B=4, Lq=Lkv=2048, D=1024, H=16 heads, Hd=64.
  q = x @ Wq + bq;  kv = context @ Wkv + bkv;  scores = q k^T / 8
  out = softmax(scores) v @ Wo + bo

Sharding: tensor-parallel over heads. Core c owns heads {2c, 2c+1} =
128 projection columns. Each core computes its heads' attention and a
rank-128 partial of the output projection; the host sums the 8 partials
(plus the constant bias terms) - no on-chip collectives.

Per-core dataflow (all matmul operands bf16, fp32 PSUM accumulation):
  phase A: qT = (Wq_c)^T-style projection producing q TRANSPOSED
           [128 headcols, rows] directly from host-pretransposed xT;
           same for kT; v in natural [rows, 64] orientation per head
           with a ones column appended (row 64 of the attn@v output
           then equals sum_k exp = softmax denominator).
  phase B: per (batch, 512-query block): for each 128-key tile,
           scoresT = kT^t @ qT via two K=64 matmuls packed into array
           row-groups (tile_position), exp on the scalar engine with
           the 1/8 scale folded in (no max subtraction: |scores| <~ 4
           for this distribution), attn@v accumulated over key tiles.
           Normalize by broadcasting 1/sumexp across partitions with a
           K=1 ones-matmul and multiplying on the vector engine.
  phase D: out_partial = outT^t @ Wo_c rows (K=128), bf16 partials to
           DRAM.

This walrus build rejects instructions with embedded semaphore waits,
so after TileContext emits the program every sync wait is hoisted into
a standalone InstEventSemaphore on the same engine (hoist_waits).
"""

import os
import time
import numpy as np
import ml_dtypes
from contextlib import ExitStack

import concourse.bass as bass
import concourse.mybir as mybir
import concourse.tile as tile
from concourse.bass_utils import run_bass_kernel_spmd

BF16 = mybir.dt.bfloat16
F32 = mybir.dt.float32
F32R = mybir.dt.float32r
AF = mybir.ActivationFunctionType

B, LQ, LKV, D, H, HD = 4, 2048, 2048, 1024, 16, 64
R = B * LQ            # 8192 query rows (flattened)
RK = B * LKV          # 8192 key rows
NCORES = 8
HC = 128              # head-columns per core (2 heads x 64)
SCALE = 1.0 / np.sqrt(HD)

QB = 512              # projection row block
QB2 = 1024            # attention query block (2 PSUM banks wide)
KT = 128              # key tile
NKT = LKV // KT       # 16 key tiles per batch
NQB = LQ // QB        # 4 query blocks per batch
DCH = D // 128        # 8 contraction chunks


def hoist_waits(nc, max_embedded=0):
    """Hoist embedded sync waits into standalone InstEventSemaphore ops."""
    uid = 0
    for fn in nc.m.functions:
        for bb in fn.blocks:
            insts = bb.instructions
            if not insts:
                continue
            new_insts = []
            changed = False
            for inst in insts:
                si = inst.sync_info
                waits = list(si.on_wait) if si is not None else []
                if len(waits) > max_embedded:
                    keep = waits[:max_embedded]
                    for w in waits[max_embedded:]:
                        uid += 1
                        new_insts.append(mybir.InstEventSemaphore(
                            name=f"EVW-{uid}",
                            engine=inst.engine,
                            sync_info=mybir.SyncInfo(on_wait=[w], on_update=[]),
                        ))
                    inst.sync_info = mybir.SyncInfo(
                        on_wait=keep,
                        on_update=list(si.on_update) if si is not None else [],
                    )
                    changed = True
                new_insts.append(inst)
            if changed:
                bb.instructions = new_insts
    return nc


def build_program():
    nc = bass.Bass()

    xT_e = nc.declare_dram_parameter("xT", [D, R], BF16, isOutput=False)
    cT_e = nc.declare_dram_parameter("cT", [D, RK], BF16, isOutput=False)
    wq_e = nc.declare_dram_parameter("wq", [D, HC], BF16, isOutput=False)
    wk_e = nc.declare_dram_parameter("wk", [D, HC], BF16, isOutput=False)
    wv_e = nc.declare_dram_parameter("wv", [D, HC], BF16, isOutput=False)
    wo_e = nc.declare_dram_parameter("wo", [HC, D], BF16, isOutput=False)
    bq_e = nc.declare_dram_parameter("bq", [HC, 1], F32, isOutput=False)
    bk_e = nc.declare_dram_parameter("bk", [HC, 1], F32, isOutput=False)
    out_e = nc.declare_dram_parameter("outp", [R, D], BF16, isOutput=True)

    xT3 = xT_e.rearrange("(o p) r -> p o r", p=128)
    cT3 = cT_e.rearrange("(o p) r -> p o r", p=128)
    wq3 = wq_e.rearrange("(o p) m -> p o m", p=128)
    wk3 = wk_e.rearrange("(o p) m -> p o m", p=128)
    wv3 = wv_e.rearrange("(o p) m -> p o m", p=128)

    with tile.TileContext(nc) as tc:
        with ExitStack() as ctx:
            consts = ctx.enter_context(tc.tile_pool(name="consts", bufs=1))
            perb = ctx.enter_context(tc.tile_pool(name="perb", bufs=4))
            outtp = ctx.enter_context(tc.tile_pool(name="outtp", bufs=2))
            stream = ctx.enter_context(tc.tile_pool(name="stream", bufs=3))
            expp = ctx.enter_context(tc.tile_pool(name="expp", bufs=4))
            smalls = ctx.enter_context(tc.tile_pool(name="smalls", bufs=2))
            outs = ctx.enter_context(tc.tile_pool(name="outs", bufs=8))
            ps_pair = ctx.enter_context(tc.tile_pool(name="ps_pair", bufs=2, space="PSUM"))
            ps_av = ctx.enter_context(tc.tile_pool(name="ps_av", bufs=1, space="PSUM"))
            ps_x = ctx.enter_context(tc.tile_pool(name="ps_x", bufs=2, space="PSUM"))

            # weights / biases
            wq_sb = consts.tile([128, DCH, HC], BF16)
            wk_sb = consts.tile([128, DCH, HC], BF16)
            wv_sb = consts.tile([128, DCH, HC], BF16)
            wo_sb = consts.tile([HC, D], BF16)
            bq_sb = consts.tile([HC, 1], F32)
            bk_sb = consts.tile([HC, 1], F32)
            nc.sync.dma_start(wq_sb[:], wq3[:])
            nc.sync.dma_start(wk_sb[:], wk3[:])
            nc.sync.dma_start(wv_sb[:], wv3[:])
            nc.sync.dma_start(wo_sb[:], wo_e[:])
            nc.sync.dma_start(bq_sb[:], bq_e[:])
            nc.sync.dma_start(bk_sb[:], bk_e[:])
            ones_sb = consts.tile([1, 64], BF16)
            nc.vector.memset(ones_sb[:], 1.0)

            for b in range(B):
                qT_b = perb.tile([128, LQ], BF16, tag="qT")
                kT_b = perb.tile([128, LKV], BF16, tag="kT")
                v_b = perb.tile([128, NKT, 144], BF16, tag="v", name="v_b")
                outT_b = outtp.tile([128, LQ], BF16, tag="outT")
                for h in (0, 1):
                    nc.vector.memset(v_b[:, :, 72 * h + 64:72 * h + 65], 1.0)

                # ---- phase A: projections for this batch ----
                for blk in range(4):
                    rbase = b * LQ + blk * QB
                    csl = slice(blk * QB, (blk + 1) * QB)
                    xt = stream.tile([128, DCH, QB], BF16, tag="xt")
                    nc.sync.dma_start(xt[:], xT3[:, :, rbase:rbase + QB])
                    psq = ps_x.tile([128, QB], F32, tag="ps_x")
                    for o in range(DCH):
                        nc.tensor.matmul(psq[:], wq_sb[:, o, :], xt[:, o, :],
                                         start=(o == 0), stop=(o == DCH - 1))
                    nc.vector.tensor_scalar_add(qT_b[:, csl], psq[:], bq_sb[:])

                    ct = stream.tile([128, DCH, QB], BF16, tag="ct")
                    nc.sync.dma_start(ct[:], cT3[:, :, rbase:rbase + QB])
                    psk = ps_x.tile([128, QB], F32, tag="ps_x")
                    for o in range(DCH):
                        nc.tensor.matmul(psk[:], wk_sb[:, o, :], ct[:, o, :],
                                         start=(o == 0), stop=(o == DCH - 1))
                    nc.vector.tensor_scalar_add(kT_b[:, csl], psk[:], bk_sb[:])

                    for rt in range(4):
                        psv = ps_x.tile([128, QB], F32, tag="ps_x")
                        for o in range(DCH):
                            nc.tensor.matmul(
                                psv[:, 0:HC],
                                ct[:, o, rt * 128:(rt + 1) * 128],
                                wv_sb[:, o, :],
                                start=(o == 0), stop=(o == DCH - 1))
                        vt = blk * 4 + rt
                        # both heads' v columns in one strided copy:
                        # psv [128, 2, 64] -> v_b[:, vt, {0:64, 72:136}]
                        nc.vector.tensor_copy(
                            out=v_b[:, vt, :].rearrange("p (g c) -> p g c", g=2)[:, :, 0:64],
                            in_=psv[:, 0:HC].rearrange("p (g c) -> p g c", g=2))

                # ---- phase B: attention ----
                for qb in range(NQB):
                    qsl = slice(qb * QB, (qb + 1) * QB)
                    av = [ps_av.tile([65, QB], F32, tag=f"av{h}", name=f"av{h}") for h in (0, 1)]
                    pend = None
                    for kt in range(NKT):
                        # both heads' scoresT into one 2-bank psum tile,
                        # one 1024-wide exp
                        ss = ps_pair.tile([128, 2, QB], F32, tag="ps_pair")
                        for h in (0, 1):
                            hp = slice(64 * h, 64 * (h + 1))
                            nc.tensor.matmul(
                                ss[:, h, :], kT_b[hp, kt * KT:(kt + 1) * KT],
                                qT_b[hp, qsl], start=True, stop=True,
                                tile_position=(64 * h, 0))
                        ex = expp.tile([128, 2, QB], BF16, tag="exp")
                        nc.scalar.activation(ex[:], ss[:], AF.Exp, scale=SCALE)
                        if pend is not None:
                            pkt, pex = pend
                            for h in (0, 1):
                                nc.tensor.matmul(
                                    av[h][:], v_b[:, pkt, 72 * h:72 * h + 65],
                                    pex[:, h, :], start=(pkt == 0), stop=False,
                                    skip_group_check=True)
                        pend = (kt, ex)
                    pkt, pex = pend
                    for h in (0, 1):
                        nc.tensor.matmul(
                            av[h][:], v_b[:, pkt, 72 * h:72 * h + 65],
                            pex[:, h, :], start=False, stop=True,
                            skip_group_check=True)
                    # normalize: outT = av[0:64] * (1/sumexp) broadcast over
                    # partitions (K=1 ones-matmul replicates the row)
                    for h in (0, 1):
                        rec = smalls.tile([1, QB], F32, tag="rec")
                        nc.vector.reciprocal(rec[:], av[h][64:65, :])
                        rec_bf = smalls.tile([1, QB], BF16, tag="recb")
                        nc.vector.tensor_copy(out=rec_bf[:], in_=rec[:])
                        bc_ps = ps_x.tile([128, QB], F32, tag="ps_x")
                        nc.tensor.matmul(bc_ps[0:64, :], ones_sb[:], rec_bf[:],
                                         start=True, stop=True)
                        bc = smalls.tile([64, QB], F32, tag="bc")
                        nc.vector.tensor_copy(out=bc[:], in_=bc_ps[0:64, :])
                        nc.vector.tensor_tensor(
                            outT_b[64 * h:64 * (h + 1), qsl],
                            av[h][0:64, :], bc[:], mybir.AluOpType.mult)

                    # ---- phase D for this query block: output projection ----
                    for t in range(4 * qb, 4 * qb + 4):
                        g = b * (LQ // 128) + t
                        for nb in range(2):
                            po = ps_x.tile([128, 512], F32, tag="ps_x")
                            nc.tensor.matmul(
                                po[:], outT_b[:, t * 128:(t + 1) * 128],
                                wo_sb[:, nb * 512:(nb + 1) * 512],
                                start=True, stop=True)
                            ot = outs.tile([128, 512], BF16, tag="o")
                            nc.vector.tensor_copy(out=ot[:], in_=po[:])
                            nc.sync.dma_start(
                                out_e[g * 128:(g + 1) * 128, nb * 512:(nb + 1) * 512],
                                ot[:])

    return hoist_waits(nc)


_PROGRAM = None


def _get_program():
    global _PROGRAM
    if _PROGRAM is None:
        _PROGRAM = build_program()
    return _PROGRAM


def kernel(x, context, Wq, bq, Wkv, bkv, Wo, bo):
    x = np.asarray(x, np.float32)
    context = np.asarray(context, np.float32)
    Wq = np.asarray(Wq, np.float32)
    bq = np.asarray(bq, np.float32)
    Wkv = np.asarray(Wkv, np.float32)
    bkv = np.asarray(bkv, np.float32)
    Wo = np.asarray(Wo, np.float32)
    bo = np.asarray(bo, np.float32)

    xT = np.ascontiguousarray(x.reshape(R, D).T).astype(ml_dtypes.bfloat16)
    cT = np.ascontiguousarray(context.reshape(RK, D).T).astype(ml_dtypes.bfloat16)
    Wk = Wkv[:, :D]
    Wv = Wkv[:, D:]

    in_maps = []
    for c in range(NCORES):
        sl = slice(HC * c, HC * (c + 1))
        in_maps.append({
            "xT": xT,
            "cT": cT,
            "wq": Wq[:, sl].astype(ml_dtypes.bfloat16),
            "wk": Wk[:, sl].astype(ml_dtypes.bfloat16),
            "wv": Wv[:, sl].astype(ml_dtypes.bfloat16),
            "wo": np.ascontiguousarray(Wo[sl, :]).astype(ml_dtypes.bfloat16),
            "bq": np.ascontiguousarray(bq[sl]).reshape(HC, 1),
            "bk": np.ascontiguousarray(bkv[:D][sl]).reshape(HC, 1),
        })

    nc = _get_program()
    t0 = time.time()
    trace = bool(int(os.environ.get("KERNEL_TRACE", "0")))
    tkw = {}
    if trace:
        tkw = dict(trace=True, tmpdir=os.environ.get("KERNEL_TRACE_DIR") or None)
    res = run_bass_kernel_spmd(nc, in_maps, list(range(NCORES)), **tkw)
    global LAST_RUN_S, LAST_RESULT
    LAST_RESULT = res
    LAST_RUN_S = time.time() - t0

    out = np.zeros((R, D), np.float32)
    for c in range(NCORES):
        out += res.results[c]["outp"].astype(np.float32)
    # constant affine terms: v-bias flows through softmax (rows sum to 1)
    # into bkv_v @ Wo, plus bo
    out += bkv[D:] @ Wo + bo
    return out.reshape(B, LQ, D).astype(np.float32)



# revision 12
# speedup vs baseline: 1.0383x; 1.0383x over previous
"""Cross-attention kernel for 8 TRN2 NeuronCores (Bass/Tile, SPMD), v2.

Problem (hardcoded): B=4, Lq=Lkv=2048, D=1024, H=16 heads, Hd=64.
  q = x @ Wq + bq;  kv = context @ Wkv + bkv;  scores = q k^T / 8
  out = softmax(scores) v @ Wo + bo

Sharding: hybrid tensor-parallel: 4 heads x 2 batches per core.
Core c owns head-group hg=c//2 (256 projection columns = 4 heads) and
batch-pair bh=c%2 (batches {2bh, 2bh+1} = 4096 query/kv rows). The host
sums the 4 partials per batch-pair (plus constant bias terms).

Per-core dataflow (matmuls bf16 except the tiny fp32 bc broadcast):
  phase A: qT,kT transposed [hc, rows] via weight-stationary matmuls
           (the 1/8 score scale and log2e are folded into Wq/bq on the
           host, so on-chip exp computes 2^t); v in natural [rows, 256]
           orientation via x-stationary matmuls (N=256 moving = the 4
           heads' v columns - this is why HC=256 beats HC=128).
  phase B: per (batch, 512-query block, head pair): for each 128-key
           tile, scoresT for both heads via two K=64 row-group-packed
           matmuls into a 2-bank PSUM tile; one FD=1024 exp on the
           scalar engine; attn@v for both heads as a col-group-packed
           concurrent matmul pair into ONE psum bank (h0 -> partitions
           0-63, h1 -> 64-127); denominators via two col-packed M=1
           ones-matmuls into a second bank (rows 0 and 32).
           PSUM has_written subtlety: only the FIRST matmul of each
           shared-bank accumulation group uses start=True (start clears
           the whole bank's accumulate bits); the second col-group's
           kt=0 matmul relies on overwrite-where-bit-unset.
           Normalize: reciprocal_approx_fast on the denominator rows,
           broadcast across partitions with a K=33 selection matmul,
           one tensor_tensor mult for both heads at once.
  phase D: out partial = outT^T @ Wo (K=256 over 2 chunks), bf16
           partials to DRAM; host sums 4 partials per batch-pair.

This walrus build rejects instructions with embedded semaphore waits,
so after TileContext emits the program every sync wait is hoisted into
a standalone InstEventSemaphore on the same engine (hoist_waits).
"""

import os
import time
import numpy as np
import ml_dtypes
from contextlib import ExitStack

import concourse.bass as bass
import concourse.mybir as mybir
import concourse.tile as tile
from concourse.bass_utils import run_bass_kernel_spmd

BF16 = mybir.dt.bfloat16
F32 = mybir.dt.float32
AF = mybir.ActivationFunctionType

B, LQ, LKV, D, H, HD = 4, 2048, 2048, 1024, 16, 64
NCORES = 8
HC = 256              # head-columns per core (4 heads)
NB = 2                # local batches per core
RL = NB * LQ          # 4096 local query rows
SCALE = 1.0 / np.sqrt(HD)
LOG2E = float(np.log2(np.e))
LN2 = float(np.log(2.0))

QB = 512              # query block
KT = 128              # key tile
NKT = LKV // KT       # 16 key tiles per batch
NQB = LQ // QB        # 4 query blocks per batch
DCH = D // 128        # 8 contraction chunks


def hoist_waits(nc, max_embedded=0):
    """Hoist embedded sync waits into standalone InstEventSemaphore ops."""
    uid = 0
    for fn in nc.m.functions:
        for bb in fn.blocks:
            insts = bb.instructions
            if not insts:
                continue
            new_insts = []
            changed = False
            for inst in insts:
                si = inst.sync_info
                waits = list(si.on_wait) if si is not None else []
                if len(waits) > max_embedded:
                    keep = waits[:max_embedded]
                    for w in waits[max_embedded:]:
                        uid += 1
                        new_insts.append(mybir.InstEventSemaphore(
                            name=f"EVW-{uid}",
                            engine=inst.engine,
                            sync_info=mybir.SyncInfo(on_wait=[w], on_update=[]),
                        ))
                    inst.sync_info = mybir.SyncInfo(
                        on_wait=keep,
                        on_update=list(si.on_update) if si is not None else [],
                    )
                    changed = True
                new_insts.append(inst)
            if changed:
                bb.instructions = new_insts
    return nc


def build_program():
    nc = bass.Bass()

    xT_e = nc.declare_dram_parameter("xT", [D, RL], BF16, isOutput=False)
    cT_e = nc.declare_dram_parameter("cT", [D, RL], BF16, isOutput=False)
    wq_e = nc.declare_dram_parameter("wq", [D, HC], BF16, isOutput=False)
    wk_e = nc.declare_dram_parameter("wk", [D, HC], BF16, isOutput=False)
    wv_e = nc.declare_dram_parameter("wv", [D, HC], BF16, isOutput=False)
    wo_e = nc.declare_dram_parameter("wo", [HC, D], BF16, isOutput=False)
    bq_e = nc.declare_dram_parameter("bq", [HC, 1], F32, isOutput=False)
    bk_e = nc.declare_dram_parameter("bk", [HC, 1], F32, isOutput=False)
    out_e = nc.declare_dram_parameter("outp", [RL, D], BF16, isOutput=True)

    xT3 = xT_e.rearrange("(o p) r -> p o r", p=128)
    cT3 = cT_e.rearrange("(o p) r -> p o r", p=128)
    wq3 = wq_e.rearrange("(o p) m -> p o m", p=128)
    wk3 = wk_e.rearrange("(o p) m -> p o m", p=128)
    wv3 = wv_e.rearrange("(o p) m -> p o m", p=128)
    wo3 = wo_e.rearrange("(t p) n -> p t n", p=128)
    bq3 = bq_e.rearrange("(t p) one -> p t one", p=128)
    bk3 = bk_e.rearrange("(t p) one -> p t one", p=128)

    with tile.TileContext(nc) as tc:
        with ExitStack() as ctx:
            consts = ctx.enter_context(tc.tile_pool(name="consts", bufs=1))
            perb = ctx.enter_context(tc.tile_pool(name="perb", bufs=2))
            stream = ctx.enter_context(tc.tile_pool(name="stream", bufs=2))
            expp = ctx.enter_context(tc.tile_pool(name="expp", bufs=5))
            outtp = ctx.enter_context(tc.tile_pool(name="outtp", bufs=2))
            recp = ctx.enter_context(tc.tile_pool(name="recp", bufs=2))
            outs = ctx.enter_context(tc.tile_pool(name="outs", bufs=4))
            ps_ss = ctx.enter_context(tc.tile_pool(name="ps_ss", bufs=3, space="PSUM"))
            ps_av = ctx.enter_context(tc.tile_pool(name="ps_av", bufs=2, space="PSUM"))
            ps_dn = ctx.enter_context(tc.tile_pool(name="ps_dn", bufs=1, space="PSUM"))
            ps_x = ctx.enter_context(tc.tile_pool(name="ps_x", bufs=2, space="PSUM"))

            # weights / biases / constants
            wq_sb = consts.tile([128, DCH, HC], BF16)
            wk_sb = consts.tile([128, DCH, HC], BF16)
            wv_sb = consts.tile([128, DCH, HC], BF16)
            wo_sb = consts.tile([128, 2, D], BF16)
            bq_sb = consts.tile([128, 2, 1], F32)
            bk_sb = consts.tile([128, 2, 1], F32)
            nc.sync.dma_start(wq_sb[:], wq3[:])
            nc.sync.dma_start(wk_sb[:], wk3[:])
            nc.sync.dma_start(wv_sb[:], wv3[:])
            nc.sync.dma_start(wo_sb[:], wo3[:])
            nc.sync.dma_start(bq_sb[:], bq3[:])
            nc.sync.dma_start(bk_sb[:], bk3[:])
            ones_sb = consts.tile([128, 1], BF16)
            nc.vector.memset(ones_sb[:], 1.0)
            # bc selection matrix: row 0 -> head0 cols, row 32 -> head1 cols
            s_sb = consts.tile([33, 128], F32)
            nc.vector.memset(s_sb[:], 0.0)
            nc.vector.memset(s_sb[0:1, 0:64], 1.0)
            nc.vector.memset(s_sb[32:33, 64:128], 1.0)
            # bank-init operands: a matmul with these writes a known value to
            # every element of a psum bank and sets all has_written bits, so
            # the per-kt accumulating matmuls can all use start=False and be
            # order-independent (start=True clears the WHOLE bank's bits,
            # which is unsafe when two col-tiled groups share a bank).
            ones512 = consts.tile([128, QB], BF16)
            nc.vector.memset(ones512[:], 1.0)
            zs_sb = consts.tile([128, 128], BF16)
            nc.vector.memset(zs_sb[:], 0.0)
            # dn-init matrix: writes 1.0 into rows 1-31 (never matmul-written,
            # read by the reciprocal - keeps them finite), 0 elsewhere
            e_sb = consts.tile([128, 128], BF16)
            nc.vector.memset(e_sb[:], 0.0)
            nc.vector.memset(e_sb[0:1, 1:32], 1.0)

            def qk_chain(dst, xt, w_sb, b_sb, t, csl):
                ps = ps_x.tile([128, QB], F32, tag="ps_x")
                for o in range(DCH):
                    nc.tensor.matmul(ps[:], w_sb[:, o, 128 * t:128 * (t + 1)],
                                     xt[:, o, :], start=(o == 0),
                                     stop=(o == DCH - 1))
                nc.vector.tensor_scalar_add(dst[:, t, csl], ps[:], b_sb[:, t, :])

            for b in range(NB):
                qT_b = perb.tile([128, 2, LQ], BF16, tag="qT")
                kT_b = perb.tile([128, 2, LQ], BF16, tag="kT")
                v_b = perb.tile([128, NKT, HC], BF16, tag="v")
                boff = b * LQ

                # ---- phase A ----
                # emission order tracks when phase B first needs each chain:
                # pair 0 of (qb=0) needs qT(qb0,t0), then per kv row-block the
                # t0 k-chain and the v chains (kt-ordered); t1 chains and the
                # remaining q blocks only matter one pair/qb later.
                def q_load(rb):
                    xt = stream.tile([128, DCH, QB], BF16, tag="xt")
                    nc.sync.dma_start(
                        xt[:], xT3[:, :, boff + rb * QB:boff + (rb + 1) * QB])
                    return xt

                def c_load(rb):
                    ct = stream.tile([128, DCH, QB], BF16, tag="ct")
                    nc.sync.dma_start(
                        ct[:], cT3[:, :, boff + rb * QB:boff + (rb + 1) * QB])
                    return ct

                def v_chains(ct, rb):
                    for rt in range(4):
                        psv = ps_x.tile([128, QB], F32, tag="ps_x")
                        for o in range(DCH):
                            nc.tensor.matmul(
                                psv[:, 0:HC],
                                ct[:, o, rt * 128:(rt + 1) * 128],
                                wv_sb[:, o, :],
                                start=(o == 0), stop=(o == DCH - 1))
                        nc.vector.tensor_copy(out=v_b[:, rb * 4 + rt, :],
                                              in_=psv[:, 0:HC])

                qk_chain(qT_b, q_load(0), wq_sb, bq_sb, 0, slice(0, QB))
                for rb in range(4):
                    rsl = slice(rb * QB, (rb + 1) * QB)
                    ct = c_load(rb)
                    qk_chain(kT_b, ct, wk_sb, bk_sb, 0, rsl)
                    v_chains(ct, rb)
                for rb in range(1, 4):
                    qk_chain(qT_b, q_load(rb), wq_sb, bq_sb, 0,
                             slice(rb * QB, (rb + 1) * QB))
                for rb in range(4):
                    rsl = slice(rb * QB, (rb + 1) * QB)
                    qk_chain(kT_b, c_load(rb), wk_sb, bk_sb, 1, rsl)
                for rb in range(4):
                    qk_chain(qT_b, q_load(rb), wq_sb, bq_sb, 1,
                             slice(rb * QB, (rb + 1) * QB))

                # ---- phase B ----
                for qb in range(NQB):
                    qsl = slice(qb * QB, (qb + 1) * QB)
                    outT_qb = outtp.tile([128, 2, QB], BF16, tag="outT")
                    for p in (0, 1):
                        av = ps_av.tile([128, QB], F32, tag="av")
                        dn = ps_dn.tile([128, QB], F32, tag="dn")

                        def init_banks():
                            # order-independent bank init (all bits set,
                            # known values); every av/dn matmul below is
                            # start=False. Emitted AFTER kt=0's scores/exp
                            # so waiting on the previous group's normalize
                            # does not block them at the PE queue head.
                            nc.tensor.matmul(av[:], zs_sb[:], ones512[:],
                                             start=True, stop=True,
                                             skip_group_check=True)
                            nc.tensor.matmul(dn[:], e_sb[:], ones512[:],
                                             start=True, stop=True,
                                             skip_group_check=True)

                        def av_dn(kt, ex, last):
                            for h in (0, 1):
                                nc.tensor.matmul(
                                    av[64 * h:64 * (h + 1), :],
                                    v_b[:, kt,
                                        128 * p + 64 * h:128 * p + 64 * (h + 1)],
                                    ex[h][:], start=False, stop=last,
                                    tile_position=(0, 64 * h),
                                    skip_group_check=True)
                            for h in (0, 1):
                                nc.tensor.matmul(
                                    dn[32 * h:32 * h + 1, :],
                                    ones_sb[:], ex[h][:],
                                    start=False, stop=last,
                                    tile_position=(0, 32 * h),
                                    skip_group_check=True)

                        # scores/exp run one kt ahead of av/dn so the tensor
                        # engine's in-order queue never stalls on the current
                        # exp (head-of-line blocking kills ACT occupancy)
                        pend = None
                        for kt in range(NKT):
                            ksl = slice(kt * KT, (kt + 1) * KT)
                            ex = []
                            for h in (0, 1):
                                hp = slice(64 * h, 64 * (h + 1))
                                ss = ps_ss.tile([128, QB], F32, tag="ss")
                                nc.tensor.matmul(
                                    ss[:], kT_b[hp, p, ksl],
                                    qT_b[hp, p, qsl], start=True, stop=True,
                                    tile_position=(64 * h, 0))
                                exh = expp.tile([128, QB], BF16, tag="ex")
                                nc.scalar.activation(exh[:], ss[:], AF.Exp,
                                                     scale=LN2)
                                ex.append(exh)
                            if pend is not None:
                                if pend[0] == 0:
                                    init_banks()
                                av_dn(pend[0], pend[1], False)
                            pend = (kt, ex)
                        av_dn(pend[0], pend[1], True)

                        # normalize; copy av out first so its bank frees
                        # immediately for the next group
                        av_sb = recp.tile([128, QB], F32, tag="avs")
                        nc.vector.tensor_copy(out=av_sb[:], in_=av[:])
                        rec = recp.tile([33, QB], F32, tag="rec")
                        nc.vector.reciprocal(rec[:], dn[0:33, :])
                        bc = ps_av.tile([128, QB], F32, tag="av")
                        nc.tensor.matmul(bc[:], s_sb[:], rec[:],
                                         start=True, stop=True)
                        nc.vector.tensor_tensor(
                            outT_qb[:, p, :], av_sb[:], bc[:],
                            mybir.AluOpType.mult)

                    # ---- phase D for this query block ----
                    for g in range(4):
                        grow = boff + qb * QB + g * 128
                        for nb2 in (0, 1):
                            po = ps_x.tile([128, QB], F32, tag="ps_x")
                            for t in (0, 1):
                                nc.tensor.matmul(
                                    po[:], outT_qb[:, t, g * 128:(g + 1) * 128],
                                    wo_sb[:, t, nb2 * 512:(nb2 + 1) * 512],
                                    start=(t == 0), stop=(t == 1))
                            ot = outs.tile([128, 512], BF16, tag="o")
                            nc.vector.tensor_copy(out=ot[:], in_=po[:])
                            nc.sync.dma_start(
                                out_e[grow:grow + 128, nb2 * 512:(nb2 + 1) * 512],
                                ot[:])

    return hoist_waits(nc)


_PROGRAM = None


def _get_program():
    global _PROGRAM
    if _PROGRAM is None:
        _PROGRAM = build_program()
    return _PROGRAM


def kernel(x, context, Wq, bq, Wkv, bkv, Wo, bo):
    x = np.asarray(x, np.float32)
    context = np.asarray(context, np.float32)
    Wq = np.asarray(Wq, np.float32)
    bq = np.asarray(bq, np.float32)
    Wkv = np.asarray(Wkv, np.float32)
    bkv = np.asarray(bkv, np.float32)
    Wo = np.asarray(Wo, np.float32)
    bo = np.asarray(bo, np.float32)

    R = B * LQ
    # fold the 1/sqrt(hd) score scale and log2(e) into the q side so the
    # on-chip activation computes exp(ln2 * t) = 2^t = e^(score/8)
    f = SCALE * LOG2E
    Wq_s = Wq * f
    bq_s = bq * f
    Wk = Wkv[:, :D]
    Wv = Wkv[:, D:]

    xT = np.ascontiguousarray(x.reshape(R, D).T).astype(ml_dtypes.bfloat16)
    cT = np.ascontiguousarray(context.reshape(R, D).T).astype(ml_dtypes.bfloat16)

    in_maps = []
    for c in range(NCORES):
        hg, bh = c // 2, c % 2
        hsl = slice(HC * hg, HC * (hg + 1))
        rsl = slice(RL * bh, RL * (bh + 1))
        in_maps.append({
            "xT": np.ascontiguousarray(xT[:, rsl]),
            "cT": np.ascontiguousarray(cT[:, rsl]),
            "wq": Wq_s[:, hsl].astype(ml_dtypes.bfloat16),
            "wk": Wk[:, hsl].astype(ml_dtypes.bfloat16),
            "wv": Wv[:, hsl].astype(ml_dtypes.bfloat16),
            "wo": np.ascontiguousarray(Wo[hsl, :]).astype(ml_dtypes.bfloat16),
            "bq": np.ascontiguousarray(bq_s[hsl]).reshape(HC, 1),
            "bk": np.ascontiguousarray(bkv[:D][hsl]).reshape(HC, 1),
        })

    nc = _get_program()
    t0 = time.time()
    trace = bool(int(os.environ.get("KERNEL_TRACE", "0")))
    tkw = {}
    if trace:
        tkw = dict(trace=True, tmpdir=os.environ.get("KERNEL_TRACE_DIR") or None)
    res = run_bass_kernel_spmd(nc, in_maps, list(range(NCORES)), **tkw)
    global LAST_RUN_S, LAST_RESULT
    LAST_RESULT = res
    LAST_RUN_S = time.time() - t0

    out = np.zeros((R, D), np.float32)
    for c in range(NCORES):
        bh = c % 2
        out[RL * bh:RL * (bh + 1)] += res.results[c]["outp"].astype(np.float32)
    # constant affine terms: v-bias flows through softmax (rows sum to 1)
    # into bkv_v @ Wo, plus bo
    out += bkv[D:] @ Wo + bo
    return out.reshape(B, LQ, D).astype(np.float32)
